# revision 4
# baseline (speedup 1.0000x reference)
"""HAN layer (4-metapath GAT + semantic attention) on 8 Trainium2 NeuronCores.

v2. Sharding: core c handles metapath m = c % 4 and node-half h = c // 4
(235 blocks of 64 nodes each; h=0 -> 64-blocks 0..234, h=1 -> 235..469).

Device per core, one NEFF:
 - phase A: feat = hs @ W (bf16, PE) -> fx in DRAM, partition-major rows
   (node u -> row (u%128)*235 + u//128) so the staged writes are 4KB/desc.
 - phase B: per 1024-edge gather group: one indirect dma_gather (512B/desc,
   descriptor-gen bound at ~2.4ns/desc on the Q7 pair), DVE builds the
   [128e, 64n] onehot (is_equal vs iota) and rhs = feat[src]*P (broadcast
   mult); per 64-node block: T accumulating matmuls (lhsT=onehot, rhs) into
   PSUM [64, 256]; ACT copies acc -> bf16 staging; 8-block staged output DMA.

Host does O(E) index prep (edge logits el/er, P = exp(leaky(el+er)) in bf16)
and the epilogue: den = segment_sum(P), out = leaky(acc/den + bias), semantic
attention. Softmax max-shift is skipped (shift-invariant, |e| small).
"""
import sys
import numpy as np

sys.path.insert(0, "/opt/trn_rl_repo")

N, E, IN, H, D = 30000, 300000, 128, 4, 64
HD = H * D                      # 256
M = 4                           # metapaths
NCORES = 8
P = 128
BN = 64                         # nodes per dst block
NPROJ = 235                     # projection blocks of 128 nodes
NPAD = NPROJ * P                # 30080
NBLK = NPAD // BN               # 470 dst blocks
NB = NBLK // 2                  # 235 blocks per core
GRP = 8                         # tiles per gather group (1024 descriptors)
OSTG = 8                        # blocks per output stage
NEG_ATTN = 0.2
NEG_ACT = 0.01

_NC_CACHE = {}


def _build_nc(Tb, nb=NB):
    """One-core program; same NEFF runs SPMD on all 8 cores."""
    import concourse.bacc as bacc
    import concourse.tile as tile
    from concourse import mybir
    from contextlib import ExitStack

    f32 = mybir.dt.float32
    bf16 = mybir.dt.bfloat16
    i16 = mybir.dt.int16
    AOP = mybir.AluOpType

    assert len(Tb) == nb
    NT = sum(Tb)
    off = [0]
    for t in Tb:
        off.append(off[-1] + t)

    nc = bacc.Bacc(num_swdge_queues=4)
    hsT = nc.declare_dram_parameter("hsT", (P, NPAD), bf16, isOutput=False)
    Wm = nc.declare_dram_parameter("Wm", (P, HD), bf16, isOutput=False)
    idx = nc.declare_dram_parameter("idx", (P, NT * 8), i16, isOutput=False)
    Pb = nc.declare_dram_parameter("Pb", (P, NT * H), bf16, isOutput=False)
    dstl = nc.declare_dram_parameter("dstl", (P, NT), bf16, isOutput=False)
    iotaf = nc.declare_dram_parameter("iotaf", (P, BN), bf16, isOutput=False)
    outp = nc.declare_dram_parameter("outp", (BN, nb * HD), bf16,
                                     isOutput=True)

    with tile.TileContext(nc) as tc, ExitStack() as ctx:
        const = ctx.enter_context(tc.tile_pool(name="const", bufs=1))
        dram = ctx.enter_context(tc.tile_pool(name="dram", bufs=1,
                                              space="DRAM"))
        ha = ctx.enter_context(tc.tile_pool(name="ha", bufs=3))
        st = ctx.enter_context(tc.tile_pool(name="st", bufs=2))
        gg = ctx.enter_context(tc.tile_pool(name="gg", bufs=8))
        ohp = ctx.enter_context(tc.tile_pool(name="ohp", bufs=7))
        rhp = ctx.enter_context(tc.tile_pool(name="rhp", bufs=4))
        ob = ctx.enter_context(tc.tile_pool(name="ob", bufs=2))
        ps = ctx.enter_context(tc.tile_pool(name="ps", bufs=4, space="PSUM"))
        psa = ctx.enter_context(tc.tile_pool(name="psa", bufs=4, space="PSUM"))

        fx = dram.tile([NPAD, HD], bf16, tag="fx")

        w_sb = const.tile([P, HD], bf16, tag="w")
        nc.sync.dma_start(out=w_sb[:], in_=Wm[:, :])
        iota_sb = const.tile([P, BN], bf16, tag="iota")
        nc.sync.dma_start(out=iota_sb[:], in_=iotaf[:, :])
        idx_sb = const.tile([P, NT * 8], i16, tag="idx")
        nc.sync.dma_start(out=idx_sb[:], in_=idx[:, :])
        pb_sb = const.tile([P, NT * H], bf16, tag="pb")
        nc.sync.dma_start(out=pb_sb[:], in_=Pb[:, :])
        dl_sb = const.tile([P, NT], bf16, tag="dl")
        nc.sync.dma_start(out=dl_sb[:], in_=dstl[:, :])

        # ---- phase A: feat = hsT^T @ W -> fx, partition-major rows ----
        SGRP = 8
        nga = -(-NPROJ // SGRP)
        fxv = fx.rearrange("(p i) d -> p i d", p=P)     # row (p, i) = node i*128+p... no: row r=p*235+i holds node i*128+p
        for g in range(nga):
            n_in_g = min(SGRP, NPROJ - g * SGRP)
            hchunk = ha.tile([P, SGRP, P], bf16, tag="ha")
            nc.sync.dma_start(
                out=hchunk[:, 0:n_in_g, :],
                in_=hsT[:, g * SGRP * P:(g * SGRP + n_in_g) * P]
                    .rearrange("p (j q) -> p j q", j=n_in_g))
            stg = st.tile([P, SGRP, HD], bf16, tag="stg")
            for j in range(n_in_g):
                pacc = psa.tile([P, HD], f32, tag="pacc")
                nc.tensor.matmul(out=pacc[:], lhsT=hchunk[:, j, :],
                                 rhs=w_sb[:], start=True, stop=True)
                nc.scalar.copy(out=stg[:, j, :], in_=pacc[:])
            nc.sync.dma_start(
                out=fxv[:, g * SGRP:g * SGRP + n_in_g, :],
                in_=stg[:, 0:n_in_g, :])

        # ---- phase B ----
        # rhs(g) waits on gather(g)'s DMA; DVE is strict FIFO, so emit rhs
        # RLAG groups behind the gather/oh stream to keep the queue head
        # unblocked (oh never depends on a gather).
        RLAG = 3
        ngrp = -(-NT // GRP)
        grp_G = {}
        grp_oh = {}
        grp_rhs = {}

        def emit_gather(g):
            k0 = g * GRP
            gl = min(GRP, NT - k0)
            G = gg.tile([P, GRP, HD], bf16, tag="G")
            nc.gpsimd.dma_gather(
                out_ap=G[:, 0:gl, :], in_ap=fx[:, :],
                idxs_ap=idx_sb[:, k0 * 8:(k0 + gl) * 8],
                num_idxs=gl * P, num_idxs_reg=gl * P, elem_size=HD,
                queue_num=g % 4)
            oh = ohp.tile([P, GRP, BN], bf16, tag="oh")
            nc.vector.tensor_tensor(
                out=oh[:, 0:gl, :],
                in0=iota_sb[:, :].unsqueeze(1).broadcast_to([P, gl, BN]),
                in1=dl_sb[:, k0:k0 + gl].unsqueeze(2).broadcast_to(
                    [P, gl, BN]),
                op=AOP.is_equal)
            grp_G[g] = G
            grp_oh[g] = oh

        def emit_rhs(g):
            k0 = g * GRP
            gl = min(GRP, NT - k0)
            G = grp_G.pop(g)
            rhs = rhp.tile([P, GRP, HD], bf16, tag="rhs")
            nc.vector.tensor_tensor(
                out=rhs[:, 0:gl, :].rearrange("p t (h d) -> p t h d", h=H),
                in0=G[:, 0:gl, :].rearrange("p t (h d) -> p t h d", h=H),
                in1=pb_sb[:, k0 * H:(k0 + gl) * H]
                    .rearrange("p (t h) -> p t h", t=gl)
                    .unsqueeze(3).broadcast_to([P, gl, H, D]),
                op=AOP.mult)
            grp_rhs[g] = rhs

        g_emitted = -1
        r_emitted = -1
        ostg = None
        for i in range(nb):
            T = Tb[i]
            o = off[i]
            need = (o + T - 1) // GRP
            while g_emitted < min(need + RLAG, ngrp - 1):
                g_emitted += 1
                emit_gather(g_emitted)
            while r_emitted < need:
                r_emitted += 1
                emit_rhs(r_emitted)
            acc = ps.tile([BN, HD], f32, tag="acc")
            for t in range(T):
                k = o + t
                oh = grp_oh[k // GRP]
                rhs = grp_rhs[k // GRP]
                nc.tensor.matmul(out=acc[:], lhsT=oh[:, k % GRP, :],
                                 rhs=rhs[:, k % GRP, :],
                                 start=(t == 0), stop=(t == T - 1))
            if i % OSTG == 0:
                ostg = ob.tile([BN, OSTG, HD], bf16, tag="ostg")
            nc.scalar.copy(out=ostg[:, i % OSTG, :], in_=acc[:])
            if i % OSTG == OSTG - 1 or i == nb - 1:
                s0 = (i // OSTG) * OSTG
                cnt = i - s0 + 1
                nc.sync.dma_start(
                    out=outp[:, s0 * HD:(s0 + cnt) * HD]
                        .rearrange("p (j d) -> p j d", j=cnt),
                    in_=ostg[:, 0:cnt, :])

    nc.compile()
    return nc


def _get_nc(Tb):
    if Tb not in _NC_CACHE:
        _NC_CACHE[Tb] = _build_nc(Tb)
    return _NC_CACHE[Tb]


def _attn_mat(a):
    """[H, D] head vectors -> [HD, H] block-diagonal matrix."""
    A = np.zeros((HD, H), np.float32)
    for h in range(H):
        A[h * D:(h + 1) * D, h] = a[h]
    return A


def _prep_metapath(hs_m, src_m, dst_m, W_m, al_m, ar_m):
    """Edge exp-weights (bf16-rounded), den, and dst-sorted edge arrays."""
    import ml_dtypes
    Wel = (W_m @ _attn_mat(al_m)).astype(np.float32)
    Wer = (W_m @ _attn_mat(ar_m)).astype(np.float32)
    el = hs_m @ Wel                                       # [N, H]
    er = hs_m @ Wer
    e = el[src_m] + er[dst_m]                             # [E, H]
    e = np.where(e > 0, e, NEG_ATTN * e)
    Pw = np.exp(e).astype(ml_dtypes.bfloat16).astype(np.float32)
    den = np.zeros((N, H), np.float32)
    np.add.at(den, dst_m, Pw)
    order = np.argsort(dst_m, kind="stable")
    ss = src_m[order].astype(np.int64)
    ds = dst_m[order].astype(np.int64)
    Ps = Pw[order]
    blk = ds // BN
    counts = np.bincount(blk, minlength=NBLK)
    starts = np.concatenate([[0], np.cumsum(counts)[:-1]])
    return ss, ds, Ps, counts, starts, den


def _pack_core(ss, ds, Ps, counts, starts, blocks, Tb):
    """Device-layout inputs for one core's block list (variable Tb)."""
    import ml_dtypes
    bf16 = ml_dtypes.bfloat16
    NT = sum(Tb)
    src_all = np.zeros(NT * P, np.int64)
    P_all = np.zeros((NT * P, H), np.float32)
    dl_all = np.zeros(NT * P, np.float32)
    o = 0
    for i, b in enumerate(blocks):
        T = Tb[i]
        c = int(counts[b])
        s0 = int(starts[b])
        sl = slice(o * P, o * P + c)
        src_all[sl] = ss[s0:s0 + c]
        P_all[sl] = Ps[s0:s0 + c]
        dl_all[sl] = ds[s0:s0 + c] - b * BN
        o += T
    # fx row remap: node u -> row (u%128)*235 + u//128
    idxv = (src_all % P) * NPROJ + src_all // P
    idx16 = np.tile(idxv.reshape(NT * 8, 16).T, (8, 1)).astype(np.int16)
    Pt = P_all.reshape(NT, P, H).transpose(1, 0, 2).reshape(P, NT * H)
    dlt = dl_all.reshape(NT, P).T
    return (np.ascontiguousarray(idx16),
            np.ascontiguousarray(Pt.astype(bf16)),
            np.ascontiguousarray(dlt.astype(bf16)))


def _run_device(hs, src, dst, W, attn_l, attn_r, bias, trace=False):
    import ml_dtypes
    from concourse.bass_utils import run_bass_kernel_spmd
    bf16 = ml_dtypes.bfloat16

    preps = [_prep_metapath(np.asarray(hs[m], np.float32), src[m], dst[m],
                            np.asarray(W[m], np.float32),
                            np.asarray(attn_l[m]), np.asarray(attn_r[m]))
             for m in range(M)]
    core_blocks = []
    for c in range(NCORES):
        h = c // M
        blocks = list(range(h * NB, (h + 1) * NB))
        counts = preps[c % M][3]
        blocks.sort(key=lambda b: int(counts[b]), reverse=True)
        core_blocks.append(blocks)
    Tb = []
    for i in range(NB):
        mx = 1
        for c in range(NCORES):
            b = core_blocks[c][i]
            mx = max(mx, -(-int(preps[c % M][3][b]) // P))
        Tb.append(mx)
    Tb = tuple(Tb)
    nc = _get_nc(Tb)

    iota = np.ascontiguousarray(
        np.tile(np.arange(BN, dtype=np.float32), (P, 1)).astype(bf16))
    in_maps = []
    for c in range(NCORES):
        m = c % M
        ss, ds, Ps, counts, starts, _den = preps[m]
        idx16, Pt, dlt = _pack_core(ss, ds, Ps, counts, starts,
                                    core_blocks[c], Tb)
        hsT = np.zeros((P, NPAD), np.float32)
        hsT[:, :N] = np.asarray(hs[m], np.float32).T
        in_maps.append({
            "hsT": np.ascontiguousarray(hsT.astype(bf16)),
            "Wm": np.ascontiguousarray(np.asarray(W[m]).astype(bf16)),
            "idx": idx16, "Pb": Pt, "dstl": dlt,
            "iotaf": iota,
        })
    kw = {}
    if trace:
        kw = dict(trace=True, trace_cores=list(range(NCORES)))
    res = run_bass_kernel_spmd(nc, in_maps, list(range(NCORES)), **kw)

    outs = []
    for m in range(M):
        acc = np.zeros((NPAD, HD), np.float32)
        for c in (m, m + 4):
            rows = np.asarray(res.results[c]["outp"],
                              dtype=np.float32).reshape(BN, NB, HD)
            bids = np.asarray(core_blocks[c])
            # node b*64+p -> rows[p, slot(b)]
            acc.reshape(NBLK, BN, HD)[bids] = rows.transpose(1, 0, 2)
        acc = acc[:N]
        den = preps[m][5]                                 # [N, H]
        outm = acc.reshape(N, H, D) / np.maximum(den, 1e-9)[:, :, None]
        outm = outm + np.asarray(_run_device._bias[m]).reshape(1, H, D)
        outm = np.where(outm > 0, outm, NEG_ACT * outm).reshape(N, HD)
        outs.append(outm.astype(np.float32))
    return outs, res


def _semantic(z, Wp1, bp1, Wp2):
    w = (np.tanh(z @ Wp1 + bp1) @ Wp2).mean(0)            # [2, 1]
    w = w - w.max()
    beta = np.exp(w) / np.exp(w).sum()
    return (beta[None] * z).sum(1)


def kernel(hs, src, dst, W, attn_l, attn_r, bias, Wp1, bp1, Wp2):
    hs = np.asarray(hs, np.float32)
    src = np.asarray(src)
    dst = np.asarray(dst)
    W = np.asarray(W, np.float32)
    _run_device._bias = np.asarray(bias, np.float32)

    outs, _ = _run_device(hs, src, dst, W, attn_l, attn_r, bias)

    Wp1 = np.asarray(Wp1, np.float32)
    bp1 = np.asarray(bp1, np.float32)
    Wp2 = np.asarray(Wp2, np.float32)
    lnc = _semantic(np.stack([outs[1], outs[2]], axis=1), Wp1, bp1, Wp2)
    dis = _semantic(np.stack([outs[0], outs[3]], axis=1), Wp1, bp1, Wp2)
    return np.stack([lnc, dis]).astype(np.float32)


# revision 5
# speedup vs baseline: 1.0973x; 1.0973x over previous
"""HAN layer (4-metapath GAT + semantic attention) on 8 Trainium2 NeuronCores.

v3. Sharding: core c handles metapath m = c % 4 and node-half h = c // 4
(235 dst blocks of 64 nodes; h=0 -> 64-blocks 0..234, h=1 -> 235..469).

Device per core, one NEFF:
 - phase A: feat = hs @ W (bf16, PE) -> fx in DRAM, partition-major rows
   (node u -> row (u%128)*235 + u//128) so staged writes are 4KB/descriptor;
   PSUM->SBUF copies batched 4 blocks per ACT instruction.
 - phase B: 1024-descriptor indirect dma_gathers (512B/desc; descriptor-gen
   bound at ~2.4ns/desc on the Q7 pair), paired into 16-tile DVE batches:
   oh = is_equal(iota, dst-local) [128e, 64n] and rhs = feat[src]*P
   (broadcast mult); per 64-node block: T accumulating matmuls
   (lhsT=onehot [128,64], rhs [128,256]) into PSUM [64, 256]; ACT copies
   acc -> bf16 staging; 8-block staged output DMA.
 - DVE is strict FIFO: rhs pairs are emitted QLAG pairs behind the gather
   stream so the queue head never waits on an in-flight gather; the first
   OHPRE oh-pairs are emitted up front and execute during phase A.

Host does O(E) index prep (edge logits el/er, P = exp(leaky(el+er)) in bf16)
and the epilogue: den = segment_sum(P), out = leaky(acc/den + bias), semantic
attention. Softmax max-shift is skipped (shift-invariant, |e| small).
"""
import sys
import numpy as np

sys.path.insert(0, "/opt/trn_rl_repo")

N, E, IN, H, D = 30000, 300000, 128, 4, 64
HD = H * D                      # 256
M = 4                           # metapaths
NCORES = 8
P = 128
BN = 64                         # nodes per dst block
NPROJ = 235                     # projection blocks of 128 nodes
NPAD = NPROJ * P                # 30080
NBLK = NPAD // BN               # 470 dst blocks
NB = NBLK // 2                  # 235 blocks per core
GRP = 8                         # tiles per gather call (1024 descriptors)
GRP2 = 2 * GRP                  # tiles per DVE batch (pair of gathers)
OHPRE = 16                      # oh-pairs pre-emitted before phase A
QLAG = 2                        # rhs pairs emitted this many pairs late
OSTG = 8                        # blocks per output stage
NEG_ATTN = 0.2
NEG_ACT = 0.01

_NC_CACHE = {}


def _build_nc(Tb, nb=NB):
    """One-core program; same NEFF runs SPMD on all 8 cores."""
    import concourse.bacc as bacc
    import concourse.tile as tile
    from concourse import mybir
    from contextlib import ExitStack

    f32 = mybir.dt.float32
    bf16 = mybir.dt.bfloat16
    i16 = mybir.dt.int16
    AOP = mybir.AluOpType

    assert len(Tb) == nb
    NT = sum(Tb)
    off = [0]
    for t in Tb:
        off.append(off[-1] + t)

    nc = bacc.Bacc(num_swdge_queues=4)
    hsT = nc.declare_dram_parameter("hsT", (P, NPAD), bf16, isOutput=False)
    Wm = nc.declare_dram_parameter("Wm", (P, HD), bf16, isOutput=False)
    idx = nc.declare_dram_parameter("idx", (P, NT * 8), i16, isOutput=False)
    Pb = nc.declare_dram_parameter("Pb", (P, NT * H), bf16, isOutput=False)
    dstl = nc.declare_dram_parameter("dstl", (P, NT), bf16, isOutput=False)
    iotaf = nc.declare_dram_parameter("iotaf", (P, BN), bf16, isOutput=False)
    outp = nc.declare_dram_parameter("outp", (BN, nb * HD), bf16,
                                     isOutput=True)

    with tile.TileContext(nc) as tc, ExitStack() as ctx:
        const = ctx.enter_context(tc.tile_pool(name="const", bufs=1))
        dram = ctx.enter_context(tc.tile_pool(name="dram", bufs=1,
                                              space="DRAM"))
        ha = ctx.enter_context(tc.tile_pool(name="ha", bufs=3))
        st = ctx.enter_context(tc.tile_pool(name="st", bufs=2))
        gpair = ctx.enter_context(tc.tile_pool(name="gpair", bufs=4))
        ohp = ctx.enter_context(tc.tile_pool(name="ohp", bufs=OHPRE))
        rhp = ctx.enter_context(tc.tile_pool(name="rhp", bufs=3))
        ob = ctx.enter_context(tc.tile_pool(name="ob", bufs=2))
        ps = ctx.enter_context(tc.tile_pool(name="ps", bufs=4, space="PSUM"))
        psa = ctx.enter_context(tc.tile_pool(name="psa", bufs=2, space="PSUM"))

        fx = dram.tile([NPAD, HD], bf16, tag="fx")

        w_sb = const.tile([P, HD], bf16, tag="w")
        nc.sync.dma_start(out=w_sb[:], in_=Wm[:, :])
        iota_sb = const.tile([P, BN], bf16, tag="iota")
        nc.sync.dma_start(out=iota_sb[:], in_=iotaf[:, :])
        idx_sb = const.tile([P, NT * 8], i16, tag="idx")
        nc.sync.dma_start(out=idx_sb[:], in_=idx[:, :])
        pb_sb = const.tile([P, NT * H], bf16, tag="pb")
        nc.sync.dma_start(out=pb_sb[:], in_=Pb[:, :])
        dl_sb = const.tile([P, NT], bf16, tag="dl")
        nc.sync.dma_start(out=dl_sb[:], in_=dstl[:, :])

        npair = -(-NT // GRP2)
        grp_G = {}
        grp_oh = {}
        grp_rhs = {}

        def emit_oh(q):
            L = min(GRP2, NT - q * GRP2)
            ohx = ohp.tile([P, GRP2, BN], bf16, tag="oh")
            nc.vector.tensor_tensor(
                out=ohx[:, 0:L, :],
                in0=iota_sb[:, :].unsqueeze(1).broadcast_to([P, L, BN]),
                in1=dl_sb[:, q * GRP2:q * GRP2 + L].unsqueeze(2)
                    .broadcast_to([P, L, BN]),
                op=AOP.is_equal)
            grp_oh[q] = ohx

        # oh builds depend only on consts; give the DVE a head start so
        # they execute under phase A.
        for q in range(min(OHPRE, npair)):
            emit_oh(q)

        # ---- phase A: feat = hsT^T @ W -> fx, partition-major rows ----
        SGRP = 8
        nga = -(-NPROJ // SGRP)
        fxv = fx.rearrange("(p i) d -> p i d", p=P)
        for g in range(nga):
            n_in_g = min(SGRP, NPROJ - g * SGRP)
            hchunk = ha.tile([P, SGRP, P], bf16, tag="ha")
            nc.sync.dma_start(
                out=hchunk[:, 0:n_in_g, :],
                in_=hsT[:, g * SGRP * P:(g * SGRP + n_in_g) * P]
                    .rearrange("p (j q) -> p j q", j=n_in_g))
            stg = st.tile([P, SGRP, HD], bf16, tag="stg")
            for j0 in range(0, n_in_g, 4):
                cnt = min(4, n_in_g - j0)
                pacc4 = psa.tile([P, 4, HD], f32, tag="pacc")
                for j in range(j0, j0 + cnt):
                    nc.tensor.matmul(out=pacc4[:, j - j0, :],
                                     lhsT=hchunk[:, j, :],
                                     rhs=w_sb[:], start=True, stop=True)
                nc.scalar.copy(out=stg[:, j0:j0 + cnt, :],
                               in_=pacc4[:, 0:cnt, :])
            nc.sync.dma_start(
                out=fxv[:, g * SGRP:g * SGRP + n_in_g, :],
                in_=stg[:, 0:n_in_g, :])

        # ---- phase B ----
        qcnt = [0]

        def emit_pair_gathers(q):
            L = min(GRP2, NT - q * GRP2)
            Gp = gpair.tile([P, GRP2, HD], bf16, tag="G")
            for half in (0, 1):
                lo = half * GRP
                hi = min(L, lo + GRP)
                if hi <= lo:
                    break
                k0 = q * GRP2 + lo
                nc.gpsimd.dma_gather(
                    out_ap=Gp[:, lo:hi, :], in_ap=fx[:, :],
                    idxs_ap=idx_sb[:, k0 * 8:(k0 + hi - lo) * 8],
                    num_idxs=(hi - lo) * P, num_idxs_reg=(hi - lo) * P,
                    elem_size=HD, queue_num=qcnt[0] % 4)
                qcnt[0] += 1
            grp_G[q] = Gp

        def emit_rhs(q):
            L = min(GRP2, NT - q * GRP2)
            Gp = grp_G.pop(q)
            rhs = rhp.tile([P, GRP2, HD], bf16, tag="rhs")
            nc.vector.tensor_tensor(
                out=rhs[:, 0:L, :].rearrange("p t (h d) -> p t h d", h=H),
                in0=Gp[:, 0:L, :].rearrange("p t (h d) -> p t h d", h=H),
                in1=pb_sb[:, q * GRP2 * H:(q * GRP2 + L) * H]
                    .rearrange("p (t h) -> p t h", t=L)
                    .unsqueeze(3).broadcast_to([P, L, H, D]),
                op=AOP.mult)
            grp_rhs[q] = rhs

        g_q = -1
        r_q = -1
        ostg = None
        for i in range(nb):
            T = Tb[i]
            o = off[i]
            need_q = (o + T - 1) // GRP2
            while g_q < min(need_q + QLAG, npair - 1):
                g_q += 1
                emit_pair_gathers(g_q)
                if g_q >= OHPRE:
                    emit_oh(g_q)
            while r_q < need_q:
                r_q += 1
                emit_rhs(r_q)
            acc = ps.tile([BN, HD], f32, tag="acc")
            for t in range(T):
                k = o + t
                oh = grp_oh[k // GRP2]
                rhs = grp_rhs[k // GRP2]
                nc.tensor.matmul(out=acc[:], lhsT=oh[:, k % GRP2, :],
                                 rhs=rhs[:, k % GRP2, :],
                                 start=(t == 0), stop=(t == T - 1))
            if i % OSTG == 0:
                ostg = ob.tile([BN, OSTG, HD], bf16, tag="ostg")
            nc.scalar.copy(out=ostg[:, i % OSTG, :], in_=acc[:])
            if i % OSTG == OSTG - 1 or i == nb - 1:
                s0 = (i // OSTG) * OSTG
                cnt = i - s0 + 1
                nc.sync.dma_start(
                    out=outp[:, s0 * HD:(s0 + cnt) * HD]
                        .rearrange("p (j d) -> p j d", j=cnt),
                    in_=ostg[:, 0:cnt, :])

    nc.compile()
    return nc


def _get_nc(Tb):
    if Tb not in _NC_CACHE:
        _NC_CACHE[Tb] = _build_nc(Tb)
    return _NC_CACHE[Tb]


def _attn_mat(a):
    """[H, D] head vectors -> [HD, H] block-diagonal matrix."""
    A = np.zeros((HD, H), np.float32)
    for h in range(H):
        A[h * D:(h + 1) * D, h] = a[h]
    return A


def _prep_metapath(hs_m, src_m, dst_m, W_m, al_m, ar_m):
    """Edge exp-weights (bf16-rounded), den, and dst-sorted edge arrays."""
    import ml_dtypes
    Wel = (W_m @ _attn_mat(al_m)).astype(np.float32)
    Wer = (W_m @ _attn_mat(ar_m)).astype(np.float32)
    el = hs_m @ Wel                                       # [N, H]
    er = hs_m @ Wer
    e = el[src_m] + er[dst_m]                             # [E, H]
    e = np.where(e > 0, e, NEG_ATTN * e)
    Pw = np.exp(e).astype(ml_dtypes.bfloat16).astype(np.float32)
    den = np.zeros((N, H), np.float32)
    np.add.at(den, dst_m, Pw)
    order = np.argsort(dst_m, kind="stable")
    ss = src_m[order].astype(np.int64)
    ds = dst_m[order].astype(np.int64)
    Ps = Pw[order]
    blk = ds // BN
    counts = np.bincount(blk, minlength=NBLK)
    starts = np.concatenate([[0], np.cumsum(counts)[:-1]])
    return ss, ds, Ps, counts, starts, den


def _pack_core(ss, ds, Ps, counts, starts, blocks, Tb):
    """Device-layout inputs for one core's block list (variable Tb)."""
    import ml_dtypes
    bf16 = ml_dtypes.bfloat16
    NT = sum(Tb)
    src_all = np.zeros(NT * P, np.int64)
    P_all = np.zeros((NT * P, H), np.float32)
    dl_all = np.zeros(NT * P, np.float32)
    o = 0
    for i, b in enumerate(blocks):
        T = Tb[i]
        c = int(counts[b])
        s0 = int(starts[b])
        sl = slice(o * P, o * P + c)
        src_all[sl] = ss[s0:s0 + c]
        P_all[sl] = Ps[s0:s0 + c]
        dl_all[sl] = ds[s0:s0 + c] - b * BN
        o += T
    # fx row remap: node u -> row (u%128)*235 + u//128
    idxv = (src_all % P) * NPROJ + src_all // P
    idx16 = np.tile(idxv.reshape(NT * 8, 16).T, (8, 1)).astype(np.int16)
    Pt = P_all.reshape(NT, P, H).transpose(1, 0, 2).reshape(P, NT * H)
    dlt = dl_all.reshape(NT, P).T
    return (np.ascontiguousarray(idx16),
            np.ascontiguousarray(Pt.astype(bf16)),
            np.ascontiguousarray(dlt.astype(bf16)))


def _run_device(hs, src, dst, W, attn_l, attn_r, bias, trace=False):
    import ml_dtypes
    from concourse.bass_utils import run_bass_kernel_spmd
    bf16 = ml_dtypes.bfloat16

    preps = [_prep_metapath(np.asarray(hs[m], np.float32), src[m], dst[m],
                            np.asarray(W[m], np.float32),
                            np.asarray(attn_l[m]), np.asarray(attn_r[m]))
             for m in range(M)]
    core_blocks = []
    for c in range(NCORES):
        h = c // M
        blocks = list(range(h * NB, (h + 1) * NB))
        counts = preps[c % M][3]
        blocks.sort(key=lambda b: int(counts[b]), reverse=True)
        core_blocks.append(blocks)
    Tb = []
    for i in range(NB):
        mx = 1
        for c in range(NCORES):
            b = core_blocks[c][i]
            mx = max(mx, -(-int(preps[c % M][3][b]) // P))
        Tb.append(mx)
    Tb = tuple(Tb)
    nc = _get_nc(Tb)

    iota = np.ascontiguousarray(
        np.tile(np.arange(BN, dtype=np.float32), (P, 1)).astype(bf16))
    in_maps = []
    for c in range(NCORES):
        m = c % M
        ss, ds, Ps, counts, starts, _den = preps[m]
        idx16, Pt, dlt = _pack_core(ss, ds, Ps, counts, starts,
                                    core_blocks[c], Tb)
        hsT = np.zeros((P, NPAD), np.float32)
        hsT[:, :N] = np.asarray(hs[m], np.float32).T
        in_maps.append({
            "hsT": np.ascontiguousarray(hsT.astype(bf16)),
            "Wm": np.ascontiguousarray(np.asarray(W[m]).astype(bf16)),
            "idx": idx16, "Pb": Pt, "dstl": dlt,
            "iotaf": iota,
        })
    kw = {}
    if trace:
        kw = dict(trace=True, trace_cores=list(range(NCORES)))
    res = run_bass_kernel_spmd(nc, in_maps, list(range(NCORES)), **kw)

    outs = []
    for m in range(M):
        acc = np.zeros((NPAD, HD), np.float32)
        for c in (m, m + 4):
            rows = np.asarray(res.results[c]["outp"],
                              dtype=np.float32).reshape(BN, NB, HD)
            bids = np.asarray(core_blocks[c])
            acc.reshape(NBLK, BN, HD)[bids] = rows.transpose(1, 0, 2)
        acc = acc[:N]
        den = preps[m][5]                                 # [N, H]
        outm = acc.reshape(N, H, D) / np.maximum(den, 1e-9)[:, :, None]
        outm = outm + np.asarray(_run_device._bias[m]).reshape(1, H, D)
        outm = np.where(outm > 0, outm, NEG_ACT * outm).reshape(N, HD)
        outs.append(outm.astype(np.float32))
    return outs, res


def _semantic(z, Wp1, bp1, Wp2):
    w = (np.tanh(z @ Wp1 + bp1) @ Wp2).mean(0)            # [2, 1]
    w = w - w.max()
    beta = np.exp(w) / np.exp(w).sum()
    return (beta[None] * z).sum(1)


def kernel(hs, src, dst, W, attn_l, attn_r, bias, Wp1, bp1, Wp2):
    hs = np.asarray(hs, np.float32)
    src = np.asarray(src)
    dst = np.asarray(dst)
    W = np.asarray(W, np.float32)
    _run_device._bias = np.asarray(bias, np.float32)

    outs, _ = _run_device(hs, src, dst, W, attn_l, attn_r, bias)

    Wp1 = np.asarray(Wp1, np.float32)
    bp1 = np.asarray(bp1, np.float32)
    Wp2 = np.asarray(Wp2, np.float32)
    lnc = _semantic(np.stack([outs[1], outs[2]], axis=1), Wp1, bp1, Wp2)
    dis = _semantic(np.stack([outs[0], outs[3]], axis=1), Wp1, bp1, Wp2)
    return np.stack([lnc, dis]).astype(np.float32)


# revision 8
# speedup vs baseline: 1.3335x; 1.2152x over previous
"""HAN layer (4-metapath GAT + semantic attention) on 8 Trainium2 NeuronCores.

v3. Sharding: core c handles metapath m = c % 4 and node-half h = c // 4
(235 dst blocks of 64 nodes; h=0 -> 64-blocks 0..234, h=1 -> 235..469).

Device per core, one NEFF:
 - phase A: feat = hs @ W (bf16, PE) -> fx in DRAM, partition-major rows
   (node u -> row (u%128)*235 + u//128) so staged writes are 4KB/descriptor;
   PSUM->SBUF copies batched 4 blocks per ACT instruction.
 - phase B: 1024-descriptor indirect dma_gathers (512B/desc; descriptor-gen
   bound at ~2.4ns/desc on the Q7 pair), paired into 16-tile DVE batches:
   oh = is_equal(iota, dst-local) [128e, 64n] and rhs = feat[src]*P
   (broadcast mult); per 64-node block: T accumulating matmuls
   (lhsT=onehot [128,64], rhs [128,256]) into PSUM [64, 256]; ACT copies
   acc -> bf16 staging; 8-block staged output DMA.
 - DVE is strict FIFO: rhs pairs are emitted QLAG pairs behind the gather
   stream so the queue head never waits on an in-flight gather; the first
   OHPRE oh-pairs are emitted up front and execute during phase A.

Host does O(E) index prep (edge logits el/er, P = exp(leaky(el+er)) in bf16)
and the epilogue: den = segment_sum(P), out = leaky(acc/den + bias), semantic
attention. Softmax max-shift is skipped (shift-invariant, |e| small).
"""
import sys
import numpy as np

sys.path.insert(0, "/opt/trn_rl_repo")

N, E, IN, H, D = 30000, 300000, 128, 4, 64
HD = H * D                      # 256
M = 4                           # metapaths
NCORES = 8
P = 128
BN = 64                         # nodes per dst block
NPROJ = 235                     # projection blocks of 128 nodes
NPAD = NPROJ * P                # 30080
NBLK = NPAD // BN               # 470 dst blocks
NB = NBLK // 2                  # 235 blocks per core
GRP = 8                         # tiles per gather call (1024 descriptors)
GRP2 = 2 * GRP                  # tiles per DVE batch (pair of gathers)
OHPRE = 28                      # oh-pairs pre-emitted before phase A
QLAG = 2                        # rhs pairs emitted this many pairs late
OSTG = 8                        # blocks per output stage
NEG_ATTN = 0.2
NEG_ACT = 0.01

_NC_CACHE = {}


def _build_nc(Tb, nb=NB):
    """One-core program; same NEFF runs SPMD on all 8 cores."""
    import concourse.bacc as bacc
    import concourse.tile as tile
    from concourse import mybir
    from contextlib import ExitStack

    f32 = mybir.dt.float32
    bf16 = mybir.dt.bfloat16
    i16 = mybir.dt.int16
    AOP = mybir.AluOpType

    assert len(Tb) == nb
    NT = sum(Tb)
    off = [0]
    for t in Tb:
        off.append(off[-1] + t)

    nc = bacc.Bacc(num_swdge_queues=4)
    hsT = nc.declare_dram_parameter("hsT", (P, NPAD), bf16, isOutput=False)
    Wm = nc.declare_dram_parameter("Wm", (P, HD), bf16, isOutput=False)
    idx = nc.declare_dram_parameter("idx", (P, NT * 8), i16, isOutput=False)
    Pb = nc.declare_dram_parameter("Pb", (P, NT * H), bf16, isOutput=False)
    dstl = nc.declare_dram_parameter("dstl", (P, NT), bf16, isOutput=False)
    iotaf = nc.declare_dram_parameter("iotaf", (P, BN), bf16, isOutput=False)
    outp = nc.declare_dram_parameter("outp", (BN, nb * HD), bf16,
                                     isOutput=True)

    with tile.TileContext(nc) as tc, ExitStack() as ctx:
        const = ctx.enter_context(tc.tile_pool(name="const", bufs=1))
        dram = ctx.enter_context(tc.tile_pool(name="dram", bufs=1,
                                              space="DRAM"))
        ha = ctx.enter_context(tc.tile_pool(name="ha", bufs=3))
        st = ctx.enter_context(tc.tile_pool(name="st", bufs=2))
        gpair = ctx.enter_context(tc.tile_pool(name="gpair", bufs=4))
        ohp = ctx.enter_context(tc.tile_pool(name="ohp", bufs=OHPRE))
        rhp = ctx.enter_context(tc.tile_pool(name="rhp", bufs=3))
        ob = ctx.enter_context(tc.tile_pool(name="ob", bufs=2))
        ps = ctx.enter_context(tc.tile_pool(name="ps", bufs=4, space="PSUM"))
        psa = ctx.enter_context(tc.tile_pool(name="psa", bufs=2, space="PSUM"))

        fx = dram.tile([NPAD, HD], bf16, tag="fx")

        w_sb = const.tile([P, HD], bf16, tag="w")
        nc.sync.dma_start(out=w_sb[:], in_=Wm[:, :])
        iota_sb = const.tile([P, BN], bf16, tag="iota")
        nc.sync.dma_start(out=iota_sb[:], in_=iotaf[:, :])
        idx_sb = const.tile([P, NT * 8], i16, tag="idx")
        nc.sync.dma_start(out=idx_sb[:], in_=idx[:, :])
        pb_sb = const.tile([P, NT * H], bf16, tag="pb")
        nc.sync.dma_start(out=pb_sb[:], in_=Pb[:, :])
        dl_sb = const.tile([P, NT], bf16, tag="dl")
        nc.sync.dma_start(out=dl_sb[:], in_=dstl[:, :])

        npair = -(-NT // GRP2)
        grp_G = {}
        grp_oh = {}
        grp_rhs = {}

        def emit_oh(q):
            L = min(GRP2, NT - q * GRP2)
            ohx = ohp.tile([P, GRP2, BN], bf16, tag="oh")
            nc.vector.tensor_tensor(
                out=ohx[:, 0:L, :],
                in0=iota_sb[:, :].unsqueeze(1).broadcast_to([P, L, BN]),
                in1=dl_sb[:, q * GRP2:q * GRP2 + L].unsqueeze(2)
                    .broadcast_to([P, L, BN]),
                op=AOP.is_equal)
            grp_oh[q] = ohx

        # oh builds depend only on consts; give the DVE a head start so
        # they execute under phase A.
        for q in range(min(OHPRE, npair)):
            emit_oh(q)

        # ---- phase A: feat = hsT^T @ W -> fx, partition-major rows ----
        SGRP = 8
        nga = -(-NPROJ // SGRP)
        fxv = fx.rearrange("(p i) d -> p i d", p=P)
        for g in range(nga):
            n_in_g = min(SGRP, NPROJ - g * SGRP)
            hchunk = ha.tile([P, SGRP, P], bf16, tag="ha")
            nc.sync.dma_start(
                out=hchunk[:, 0:n_in_g, :],
                in_=hsT[:, g * SGRP * P:(g * SGRP + n_in_g) * P]
                    .rearrange("p (j q) -> p j q", j=n_in_g))
            stg = st.tile([P, SGRP, HD], bf16, tag="stg")
            for j0 in range(0, n_in_g, 4):
                cnt = min(4, n_in_g - j0)
                pacc4 = psa.tile([P, 4, HD], f32, tag="pacc")
                for j in range(j0, j0 + cnt):
                    nc.tensor.matmul(out=pacc4[:, j - j0, :],
                                     lhsT=hchunk[:, j, :],
                                     rhs=w_sb[:], start=True, stop=True)
                if (g + j0 // 4) % 2 == 0:
                    nc.scalar.copy(out=stg[:, j0:j0 + cnt, :],
                                   in_=pacc4[:, 0:cnt, :])
                else:
                    nc.vector.tensor_copy(out=stg[:, j0:j0 + cnt, :],
                                          in_=pacc4[:, 0:cnt, :])
            nc.sync.dma_start(
                out=fxv[:, g * SGRP:g * SGRP + n_in_g, :],
                in_=stg[:, 0:n_in_g, :])

        # ---- phase B ----
        qcnt = [0]

        def emit_pair_gathers(q):
            L = min(GRP2, NT - q * GRP2)
            Gp = gpair.tile([P, GRP2, HD], bf16, tag="G")
            for half in (0, 1):
                lo = half * GRP
                hi = min(L, lo + GRP)
                if hi <= lo:
                    break
                k0 = q * GRP2 + lo
                nc.gpsimd.dma_gather(
                    out_ap=Gp[:, lo:hi, :], in_ap=fx[:, :],
                    idxs_ap=idx_sb[:, k0 * 8:(k0 + hi - lo) * 8],
                    num_idxs=(hi - lo) * P, num_idxs_reg=(hi - lo) * P,
                    elem_size=HD, queue_num=qcnt[0] % 4)
                qcnt[0] += 1
            grp_G[q] = Gp

        def emit_rhs(q):
            L = min(GRP2, NT - q * GRP2)
            Gp = grp_G.pop(q)
            rhs = rhp.tile([P, GRP2, HD], bf16, tag="rhs")
            nc.vector.tensor_tensor(
                out=rhs[:, 0:L, :].rearrange("p t (h d) -> p t h d", h=H),
                in0=Gp[:, 0:L, :].rearrange("p t (h d) -> p t h d", h=H),
                in1=pb_sb[:, q * GRP2 * H:(q * GRP2 + L) * H]
                    .rearrange("p (t h) -> p t h", t=L)
                    .unsqueeze(3).broadcast_to([P, L, H, D]),
                op=AOP.mult)
            grp_rhs[q] = rhs

        g_q = -1
        r_q = -1
        ostg = None
        for i in range(nb):
            T = Tb[i]
            o = off[i]
            need_q = (o + T - 1) // GRP2
            while g_q < min(need_q + QLAG, npair - 1):
                g_q += 1
                emit_pair_gathers(g_q)
                if g_q >= OHPRE:
                    emit_oh(g_q)
            while r_q < need_q:
                r_q += 1
                emit_rhs(r_q)
            acc = ps.tile([BN, HD], f32, tag="acc")
            for t in range(T):
                k = o + t
                oh = grp_oh[k // GRP2]
                rhs = grp_rhs[k // GRP2]
                nc.tensor.matmul(out=acc[:], lhsT=oh[:, k % GRP2, :],
                                 rhs=rhs[:, k % GRP2, :],
                                 start=(t == 0), stop=(t == T - 1))
            if i % OSTG == 0:
                ostg = ob.tile([BN, OSTG, HD], bf16, tag="ostg")
            nc.scalar.copy(out=ostg[:, i % OSTG, :], in_=acc[:])
            if i % OSTG == OSTG - 1 or i == nb - 1:
                s0 = (i // OSTG) * OSTG
                cnt = i - s0 + 1
                nc.sync.dma_start(
                    out=outp[:, s0 * HD:(s0 + cnt) * HD]
                        .rearrange("p (j d) -> p j d", j=cnt),
                    in_=ostg[:, 0:cnt, :])

    nc.compile()
    return nc


def _get_nc(Tb):
    if Tb not in _NC_CACHE:
        _NC_CACHE[Tb] = _build_nc(Tb)
    return _NC_CACHE[Tb]


def _attn_mat(a):
    """[H, D] head vectors -> [HD, H] block-diagonal matrix."""
    A = np.zeros((HD, H), np.float32)
    for h in range(H):
        A[h * D:(h + 1) * D, h] = a[h]
    return A


def _prep_metapath(hs_m, src_m, dst_m, W_m, al_m, ar_m):
    """Edge exp-weights (bf16-rounded), den, and dst-sorted edge arrays."""
    import ml_dtypes
    Wel = (W_m @ _attn_mat(al_m)).astype(np.float32)
    Wer = (W_m @ _attn_mat(ar_m)).astype(np.float32)
    el = hs_m @ Wel                                       # [N, H]
    er = hs_m @ Wer
    e = el[src_m] + er[dst_m]                             # [E, H]
    e = np.where(e > 0, e, NEG_ATTN * e)
    Pw = np.exp(e).astype(ml_dtypes.bfloat16).astype(np.float32)
    den = np.zeros((N, H), np.float32)
    np.add.at(den, dst_m, Pw)
    order = np.argsort(dst_m, kind="stable")
    ss = src_m[order].astype(np.int64)
    ds = dst_m[order].astype(np.int64)
    Ps = Pw[order]
    blk = ds // BN
    counts = np.bincount(blk, minlength=NBLK)
    starts = np.concatenate([[0], np.cumsum(counts)[:-1]])
    return ss, ds, Ps, counts, starts, den


def _pack_core(ss, ds, Ps, counts, starts, blocks, Tb):
    """Device-layout inputs for one core's block list (variable Tb)."""
    import ml_dtypes
    bf16 = ml_dtypes.bfloat16
    NT = sum(Tb)
    src_all = np.zeros(NT * P, np.int64)
    P_all = np.zeros((NT * P, H), np.float32)
    dl_all = np.zeros(NT * P, np.float32)
    o = 0
    for i, b in enumerate(blocks):
        T = Tb[i]
        c = int(counts[b])
        s0 = int(starts[b])
        sl = slice(o * P, o * P + c)
        seg = np.argsort(ss[s0:s0 + c], kind="stable")
        src_all[sl] = ss[s0:s0 + c][seg]
        P_all[sl] = Ps[s0:s0 + c][seg]
        dl_all[sl] = (ds[s0:s0 + c] - b * BN)[seg]
        o += T
    # fx row remap: node u -> row (u%128)*235 + u//128
    idxv = (src_all % P) * NPROJ + src_all // P
    idx16 = np.tile(idxv.reshape(NT * 8, 16).T, (8, 1)).astype(np.int16)
    Pt = P_all.reshape(NT, P, H).transpose(1, 0, 2).reshape(P, NT * H)
    dlt = dl_all.reshape(NT, P).T
    return (np.ascontiguousarray(idx16),
            np.ascontiguousarray(Pt.astype(bf16)),
            np.ascontiguousarray(dlt.astype(bf16)))


def _run_device(hs, src, dst, W, attn_l, attn_r, bias, trace=False):
    import ml_dtypes
    from concourse.bass_utils import run_bass_kernel_spmd
    bf16 = ml_dtypes.bfloat16

    preps = [_prep_metapath(np.asarray(hs[m], np.float32), src[m], dst[m],
                            np.asarray(W[m], np.float32),
                            np.asarray(attn_l[m]), np.asarray(attn_r[m]))
             for m in range(M)]
    core_blocks = []
    for c in range(NCORES):
        h = c // M
        blocks = list(range(h * NB, (h + 1) * NB))
        counts = preps[c % M][3]
        blocks.sort(key=lambda b: int(counts[b]), reverse=True)
        core_blocks.append(blocks)
    Tb = []
    for i in range(NB):
        mx = 1
        for c in range(NCORES):
            b = core_blocks[c][i]
            mx = max(mx, -(-int(preps[c % M][3][b]) // P))
        Tb.append(mx)
    Tb = tuple(Tb)
    nc = _get_nc(Tb)

    iota = np.ascontiguousarray(
        np.tile(np.arange(BN, dtype=np.float32), (P, 1)).astype(bf16))
    in_maps = []
    for c in range(NCORES):
        m = c % M
        ss, ds, Ps, counts, starts, _den = preps[m]
        idx16, Pt, dlt = _pack_core(ss, ds, Ps, counts, starts,
                                    core_blocks[c], Tb)
        hsT = np.zeros((P, NPAD), np.float32)
        hsT[:, :N] = np.asarray(hs[m], np.float32).T
        in_maps.append({
            "hsT": np.ascontiguousarray(hsT.astype(bf16)),
            "Wm": np.ascontiguousarray(np.asarray(W[m]).astype(bf16)),
            "idx": idx16, "Pb": Pt, "dstl": dlt,
            "iotaf": iota,
        })
    kw = {}
    if trace:
        kw = dict(trace=True, trace_cores=list(range(NCORES)))
    res = run_bass_kernel_spmd(nc, in_maps, list(range(NCORES)), **kw)

    outs = []
    for m in range(M):
        acc = np.zeros((NPAD, HD), np.float32)
        for c in (m, m + 4):
            rows = np.asarray(res.results[c]["outp"],
                              dtype=np.float32).reshape(BN, NB, HD)
            bids = np.asarray(core_blocks[c])
            acc.reshape(NBLK, BN, HD)[bids] = rows.transpose(1, 0, 2)
        acc = acc[:N]
        den = preps[m][5]                                 # [N, H]
        outm = acc.reshape(N, H, D) / np.maximum(den, 1e-9)[:, :, None]
        outm = outm + np.asarray(_run_device._bias[m]).reshape(1, H, D)
        outm = np.where(outm > 0, outm, NEG_ACT * outm).reshape(N, HD)
        outs.append(outm.astype(np.float32))
    return outs, res


def _semantic(z, Wp1, bp1, Wp2):
    w = (np.tanh(z @ Wp1 + bp1) @ Wp2).mean(0)            # [2, 1]
    w = w - w.max()
    beta = np.exp(w) / np.exp(w).sum()
    return (beta[None] * z).sum(1)


def kernel(hs, src, dst, W, attn_l, attn_r, bias, Wp1, bp1, Wp2):
    hs = np.asarray(hs, np.float32)
    src = np.asarray(src)
    dst = np.asarray(dst)
    W = np.asarray(W, np.float32)
    _run_device._bias = np.asarray(bias, np.float32)

    outs, _ = _run_device(hs, src, dst, W, attn_l, attn_r, bias)

    Wp1 = np.asarray(Wp1, np.float32)
    bp1 = np.asarray(bp1, np.float32)
    Wp2 = np.asarray(Wp2, np.float32)
    lnc = _semantic(np.stack([outs[1], outs[2]], axis=1), Wp1, bp1, Wp2)
    dis = _semantic(np.stack([outs[0], outs[3]], axis=1), Wp1, bp1, Wp2)
    return np.stack([lnc, dis]).astype(np.float32)
